# revision 3
# baseline (speedup 1.0000x reference)
"""CharBiLSTM embedder on 8 Trainium2 NeuronCores (Bass/Tile).

Strategy
--------
Data-parallel over words (16384 words -> 2048/core, stratified by length so
all cores see the same length profile). Per core:

- Words are sorted ascending by length and split into blocks of W=512
  columns. The LSTM state is kept feature-major: h/c tiles are
  [128 hidden, words] so the recurrent matmul (contraction over H=128) maps
  directly onto the PE without transposes.
- Raggedness: at step t only words with len > t (a suffix of the sorted
  block) are computed; matmul/activation/vector op widths shrink with t.
- Both directions are forward scans: the host reverses each word's chars for
  the backward direction, so both scans process valid chars first; the final
  state of a word is its h at step len-1, which is streamed to HBM as part
  of the h history and picked out host-side.
- The char-embedding lookup + input projection + bias runs on the PE as
  one-hot matmuls: gates += G[char] where G = emb_table @ Wih^T + bias is a
  [256, 512] table held as matmul weights; the host uploads the one-hot
  encoding of the (ragged) char stream as two [128, n] bf16 planes, and two
  K=128 matmuls per gate chunk accumulate the input contribution straight
  into the gate PSUM alongside the Whh recurrence matmul.
- Gates: 4 PSUM chunks (order i,f,o,g) so one Sigmoid activation covers
  i,f,o in a single strided read and one Tanh covers g.

The kernel is JIT-specialized to the *structure* implied by `lengths`
(per-step active widths, block step counts); char values and the selection
of final states are runtime data.
"""

import os
import sys

sys.path.insert(0, "/opt/trn_rl_repo")

import numpy as np
import ml_dtypes

import concourse.bass as bass
import concourse.bacc as bacc
import concourse.tile as tile
import concourse.mybir as mybir
from concourse.bass_utils import run_bass_kernel_spmd

V, E, H = 256, 64, 128
N, L = 16384, 24
NCORES = 8
NPC = N // NCORES          # words per core
W = 512                    # words per block (one PSUM bank per gate chunk)
NBLK = NPC // W
FP32 = mybir.dt.float32
BF16 = mybir.dt.bfloat16
AF = mybir.ActivationFunctionType
OP = mybir.AluOpType
BF16NP = ml_dtypes.bfloat16


def _build_structure(lens_max):
    """Uniform (across cores) control structure from per-rank max lengths."""
    st = {"blocks": [], "TOT": 0}
    for b in range(NBLK):
        bl = lens_max[b * W:(b + 1) * W]
        lmax = int(bl[-1])
        steps = []
        off = 0
        for t in range(lmax):
            a_exact = int(np.searchsorted(bl, t, side="right"))  # count len<=t
            a4 = a_exact & ~3
            steps.append({"t": t, "a4": a4, "A": W - a4, "off": off})
            off += W - a4
        st["blocks"].append({"lmax": lmax, "steps": steps, "total": off,
                             "base": st["TOT"]})
        st["TOT"] += off
    st["MAXTOT"] = max((blk["total"] for blk in st["blocks"]), default=1)
    return st


def _build_program(st):
    nc = bacc.Bacc("TRN2")
    TOT, MAXTOT = st["TOT"], st["MAXTOT"]

    # weights: 24 chunks of [128, 128]: WH f(4) b(4); G f half0(4) half1(4);
    # G b half0(4) half1(4)
    w_d = nc.dram_tensor("wts", [128, 24 * 128], BF16, kind="ExternalInput")
    oh_d = {d: nc.dram_tensor(f"oh_{d}", [128, 2, max(TOT, 1)], BF16,
                              kind="ExternalInput") for d in "fb"}
    hist_d = {d: nc.dram_tensor(f"hist_{d}", [128, max(TOT, 1)], BF16,
                                kind="ExternalOutput") for d in "fb"}

    with tile.TileContext(nc) as tc:
        with (
            tc.tile_pool(name="const", bufs=1) as const_p,
            tc.tile_pool(name="oh", bufs=4) as oh_p,
            tc.tile_pool(name="hist", bufs=2) as hist_p,
            tc.tile_pool(name="state", bufs=1) as state_p,
            tc.tile_pool(name="ps", bufs=1, space="PSUM") as ps_p,
        ):
            w_sb = const_p.tile([128, 24 * 128], BF16)
            nc.sync.dma_start(w_sb[:], w_d[:])
            ch = [w_sb[:, c * 128:(c + 1) * 128] for c in range(24)]
            WH = {"f": ch[0:4], "b": ch[4:8]}
            G0 = {"f": ch[8:12], "b": ch[16:20]}
            G1 = {"f": ch[12:16], "b": ch[20:24]}

            for b in range(NBLK):
                blk = st["blocks"][b]
                if blk["lmax"] == 0:
                    continue
                hist, c_t, th, tig, gsb = {}, {}, {}, {}, {}
                for d in "fb":
                    hist[d] = hist_p.tile([128, MAXTOT], BF16, tag=f"hist_{d}",
                                          name=f"hist_{d}{b}")
                    c_t[d] = state_p.tile([128, W], FP32, tag=f"c_{d}",
                                          name=f"c_{d}{b}")
                    th[d] = state_p.tile([128, W], BF16, tag=f"th_{d}",
                                         name=f"th_{d}{b}")
                    tig[d] = state_p.tile([128, W], BF16, tag=f"tig_{d}",
                                          name=f"tig_{d}{b}")
                    gsb[d] = state_p.tile([128, 4, W], BF16, tag=f"g_{d}",
                                          name=f"g_{d}{b}")
                for t in range(blk["lmax"]):
                    s = blk["steps"][t]
                    a4, A, off = s["a4"], s["A"], s["off"]
                    gbase = blk["base"] + off
                    for d in "fb":
                        oh = oh_p.tile([128, 2, W], BF16, tag=f"oh_{d}",
                                       name=f"oh_{d}{b}_{t}")
                        nc.sync.dma_start(oh[:, :, 0:A],
                                          oh_d[d][:, :, gbase:gbase + A])
                        ps = ps_p.tile([128, 4, W], FP32, tag=f"ps_{d}",
                                       name=f"ps_{d}{b}_{t}")
                        ro0 = oh[:, 0, 0:A]
                        ro1 = oh[:, 1, 0:A]
                        if t > 0:
                            sp = blk["steps"][t - 1]
                            hoff = sp["off"] + (a4 - sp["a4"])
                            rh = hist[d][:, hoff:hoff + A]
                            for j in range(4):
                                nc.tensor.matmul(ps[:, j, a4:W], WH[d][j], rh,
                                                 start=True, stop=False)
                                nc.tensor.matmul(ps[:, j, a4:W], G0[d][j], ro0,
                                                 start=False, stop=False)
                                nc.tensor.matmul(ps[:, j, a4:W], G1[d][j], ro1,
                                                 start=False, stop=True)
                        else:
                            for j in range(4):
                                nc.tensor.matmul(ps[:, j, a4:W], G0[d][j], ro0,
                                                 start=True, stop=False)
                                nc.tensor.matmul(ps[:, j, a4:W], G1[d][j], ro1,
                                                 start=False, stop=True)
                        # i,f,o sigmoid in one strided shot; g tanh
                        nc.scalar.activation(gsb[d][:, 0:3, a4:W],
                                             ps[:, 0:3, a4:W], AF.Sigmoid)
                        nc.scalar.activation(gsb[d][:, 3, a4:W],
                                             ps[:, 3, a4:W], AF.Tanh)
                        i_s = gsb[d][:, 0, a4:W]
                        f_s = gsb[d][:, 1, a4:W]
                        o_s = gsb[d][:, 2, a4:W]
                        g_s = gsb[d][:, 3, a4:W]
                        c_s = c_t[d][:, a4:W]
                        if t > 0:
                            nc.vector.tensor_tensor(tig[d][:, a4:W], i_s, g_s,
                                                    op=OP.mult)
                            nc.vector.tensor_tensor(c_s, c_s, f_s, op=OP.mult)
                            nc.vector.tensor_tensor(c_s, c_s, tig[d][:, a4:W],
                                                    op=OP.add)
                        else:
                            nc.vector.tensor_tensor(c_s, i_s, g_s, op=OP.mult)
                        nc.scalar.activation(th[d][:, a4:W], c_s, AF.Tanh)
                        nc.vector.tensor_tensor(hist[d][:, off:off + A], o_s,
                                                th[d][:, a4:W], op=OP.mult)
                for d in "fb":
                    nc.sync.dma_start(
                        hist_d[d][:, blk["base"]:blk["base"] + blk["total"]],
                        hist[d][:, 0:blk["total"]])
    nc.compile()
    return nc


def kernel(char_indices, lengths, emb_table, Wih_f, Whh_f, bih_f, bhh_f,
           Wih_b, Whh_b, bih_b, bhh_b):
    char_indices = np.asarray(char_indices).astype(np.int32)
    lengths = np.asarray(lengths).astype(np.int32)

    # --- stratified length-sorted word assignment ---
    order = np.argsort(lengths, kind="stable")
    perm = [order[k::NCORES] for k in range(NCORES)]      # per-core word ids
    lens = [lengths[p] for p in perm]                     # ascending each
    lens_max = np.max(np.stack(lens), axis=0)
    st = _build_structure(lens_max)
    TOT = st["TOT"]

    # --- per-core char arrays (fwd and per-word-reversed bwd) ---
    cf, cb = [], []
    pos = np.arange(L)[None, :]
    for k in range(NCORES):
        chw = char_indices[perm[k]]                       # [NPC, L]
        lk = lens[k]
        valid = pos < lk[:, None]
        f = np.where(valid, chw, 0)
        rev_idx_c = np.clip(lk[:, None] - 1 - pos, 0, L - 1)
        bwd = np.where(valid, np.take_along_axis(chw, rev_idx_c, axis=1), 0)
        cf.append(f)
        cb.append(bwd)

    # --- one-hot char planes [128, 2, TOT] per (core, dir) ---
    # column gbase+(w-a4) of step (b,t) is word col w's char one-hot:
    # plane c//128, row c%128.
    oh_np = {d: [np.zeros((128, 2, max(TOT, 1)), BF16NP) for _ in range(NCORES)]
             for d in "fb"}
    for b in range(NBLK):
        blk = st["blocks"][b]
        for s in blk["steps"]:
            t, a4, A = s["t"], s["a4"], s["A"]
            gbase = blk["base"] + s["off"]
            cols = np.arange(b * W + a4, (b + 1) * W)     # word columns
            ccol = np.arange(A) + gbase
            for k in range(NCORES):
                for d, arr in (("f", cf[k]), ("b", cb[k])):
                    cvals = arr[cols, t]                  # [A]
                    oh_np[d][k][cvals % 128, cvals // 128, ccol] = 1.0

    # --- weights ---
    emb = np.asarray(emb_table, np.float32)

    def packw(Whh, Wih, bih, bhh):
        rows = [slice(0, 128), slice(128, 256), slice(384, 512), slice(256, 384)]
        wh = [np.ascontiguousarray(np.asarray(Whh, np.float32)[r].T) for r in rows]
        G = emb @ np.asarray(Wih, np.float32).T \
            + (np.asarray(bih, np.float32) + np.asarray(bhh, np.float32))[None, :]
        # lhsT[k, m] = G[half*128+k, crow_m]  -> G[half rows, crow cols]
        g0 = [G[0:128, r] for r in rows]
        g1 = [G[128:256, r] for r in rows]
        return wh, g0, g1

    wh_f, g0_f, g1_f = packw(Whh_f, Wih_f, bih_f, bhh_f)
    wh_b, g0_b, g1_b = packw(Whh_b, Wih_b, bih_b, bhh_b)
    w_all = np.concatenate(wh_f + wh_b + g0_f + g1_f + g0_b + g1_b,
                           axis=1).astype(BF16NP)

    nc = _build_program(st)
    in_maps = []
    for k in range(NCORES):
        in_maps.append({
            "wts": w_all,
            "oh_f": oh_np["f"][k], "oh_b": oh_np["b"][k],
        })
    trace = os.environ.get("LSTM_TRACE") == "1"
    res = run_bass_kernel_spmd(nc, in_maps, core_ids=list(range(NCORES)),
                               trace=trace)
    if trace and res.exec_time_ns is not None:
        print(f"HW exec time: {res.exec_time_ns} ns")
        print(f"HW exec time mean: {res.mean_exec_time_ns} ns")
        if res.instructions_and_trace:
            print(f"trace: {res.instructions_and_trace[1]}")

    # --- host-side pick of final states ---
    out = np.zeros((N, 2 * H), np.float32)
    for k in range(NCORES):
        lk = lens[k]
        nz = np.nonzero(lk > 0)[0]
        if len(nz) == 0:
            continue
        r = nz
        bidx = r // W
        tstar = lk[r] - 1
        base = np.array([st["blocks"][b]["base"] for b in bidx])
        off = np.array([st["blocks"][b]["steps"][t]["off"]
                        for b, t in zip(bidx, tstar)])
        a4 = np.array([st["blocks"][b]["steps"][t]["a4"]
                       for b, t in zip(bidx, tstar)])
        posn = base + off + (r % W - a4)
        hf = np.asarray(res.results[k]["hist_f"]).astype(np.float32)
        hb = np.asarray(res.results[k]["hist_b"]).astype(np.float32)
        out[perm[k][r], 0:H] = hf[:, posn].T
        out[perm[k][r], H:2 * H] = hb[:, posn].T
    return out


# revision 6
# speedup vs baseline: 1.1544x; 1.1544x over previous
"""CharBiLSTM embedder on 8 Trainium2 NeuronCores (Bass/Tile).

Strategy
--------
Data-parallel over words (16384 words -> 2048/core, stratified by length so
all cores see the same length profile). Per core:

- Words are sorted ascending by length and split into blocks of W=512
  columns. The LSTM state is kept feature-major: h/c tiles are
  [128 hidden, words] so the recurrent matmul (contraction over H=128) maps
  directly onto the PE without transposes.
- Raggedness: at step t only words with len > t (a suffix of the sorted
  block) are computed; matmul/activation/vector op widths shrink with t.
- Both directions are forward scans: the host reverses each word's chars for
  the backward direction, so both scans process valid chars first; the final
  state of a word is its h at step len-1, which is streamed to HBM as part
  of the h history and picked out host-side.
- The char-embedding lookup + input projection + bias runs on the PE as
  one-hot matmuls: gates += G[char] where G = emb_table @ Wih^T + bias is a
  [256, 512] table held as matmul weights; the host uploads the one-hot
  encoding of the (ragged) char stream as two [128, n] bf16 planes, and two
  K=128 matmuls per gate chunk accumulate the input contribution straight
  into the gate PSUM alongside the Whh recurrence matmul.
- Gates: 4 PSUM chunks (order i,f,o,g) so one Sigmoid activation covers
  i,f,o in a single strided read and one Tanh covers g.

The kernel is JIT-specialized to the *structure* implied by `lengths`
(per-step active widths, block step counts); char values and the selection
of final states are runtime data.
"""

import os
import sys

sys.path.insert(0, "/opt/trn_rl_repo")

import numpy as np
import ml_dtypes

import concourse.bass as bass
import concourse.bacc as bacc
import concourse.tile as tile
import concourse.mybir as mybir
from concourse.bass_utils import run_bass_kernel_spmd

V, E, H = 256, 64, 128
N, L = 16384, 24
NCORES = 8
NPC = N // NCORES          # words per core
W = 512                    # words per block (one PSUM bank per gate chunk)
NBLK = NPC // W
FP32 = mybir.dt.float32
BF16 = mybir.dt.bfloat16
AF = mybir.ActivationFunctionType
OP = mybir.AluOpType
BF16NP = ml_dtypes.bfloat16


def _build_structure(lens_max):
    """Uniform (across cores) control structure from per-rank max lengths."""
    st = {"blocks": [], "TOT": 0}
    for b in range(NBLK):
        bl = lens_max[b * W:(b + 1) * W]
        lmax = int(bl[-1])
        steps = []
        off = 0
        for t in range(lmax):
            a_exact = int(np.searchsorted(bl, t, side="right"))  # count len<=t
            a4 = a_exact & ~3
            steps.append({"t": t, "a4": a4, "A": W - a4, "off": off})
            off += W - a4
        st["blocks"].append({"lmax": lmax, "steps": steps, "total": off,
                             "base": st["TOT"]})
        st["TOT"] += off
    st["MAXTOT"] = max((blk["total"] for blk in st["blocks"]), default=1)
    return st


def _build_program(st):
    nc = bacc.Bacc("TRN2")
    TOT, MAXTOT = st["TOT"], st["MAXTOT"]

    # weights: 24 chunks of [128, 128]: WH f(4) b(4); G f half0(4) half1(4);
    # G b half0(4) half1(4)
    w_d = nc.dram_tensor("wts", [128, 24 * 128], BF16, kind="ExternalInput")
    oh_d = {d: nc.dram_tensor(f"oh_{d}", [128, 2, max(TOT, 1)], BF16,
                              kind="ExternalInput") for d in "fb"}
    hist_d = {d: nc.dram_tensor(f"hist_{d}", [128, max(TOT, 1)], BF16,
                                kind="ExternalOutput") for d in "fb"}

    with tile.TileContext(nc) as tc:
        with (
            tc.tile_pool(name="const", bufs=1) as const_p,
            tc.tile_pool(name="oh", bufs=4) as oh_p,
            tc.tile_pool(name="hist", bufs=2) as hist_p,
            tc.tile_pool(name="state", bufs=1) as state_p,
            tc.tile_pool(name="ps", bufs=1, space="PSUM") as ps_p,
        ):
            w_sb = const_p.tile([128, 24 * 128], BF16)
            nc.sync.dma_start(w_sb[:], w_d[:])
            ch = [w_sb[:, c * 128:(c + 1) * 128] for c in range(24)]
            WH = {"f": ch[0:4], "b": ch[4:8]}
            G0 = {"f": ch[8:12], "b": ch[16:20]}
            G1 = {"f": ch[12:16], "b": ch[20:24]}

            for b in range(NBLK):
                blk = st["blocks"][b]
                if blk["lmax"] == 0:
                    continue
                hist, c_t, th, tig, gsb = {}, {}, {}, {}, {}
                for d in "fb":
                    hist[d] = hist_p.tile([128, MAXTOT], BF16, tag=f"hist_{d}",
                                          name=f"hist_{d}{b}")
                    c_t[d] = state_p.tile([128, W], BF16, tag=f"c_{d}",
                                          name=f"c_{d}{b}")
                    th[d] = state_p.tile([128, W], BF16, tag=f"th_{d}",
                                         name=f"th_{d}{b}")
                    tig[d] = state_p.tile([128, W], BF16, tag=f"tig_{d}",
                                          name=f"tig_{d}{b}")
                    gsb[d] = state_p.tile([128, 4, W], BF16, tag=f"g_{d}",
                                          name=f"g_{d}{b}")
                for t in range(blk["lmax"]):
                    s = blk["steps"][t]
                    a4, A, off = s["a4"], s["A"], s["off"]
                    gbase = blk["base"] + off
                    for d in "fb":
                        oh = oh_p.tile([128, 2, W], BF16, tag=f"oh_{d}",
                                       name=f"oh_{d}{b}_{t}")
                        nc.sync.dma_start(oh[:, :, 0:A],
                                          oh_d[d][:, :, gbase:gbase + A])
                        ps = ps_p.tile([128, 4, W], FP32, tag=f"ps_{d}",
                                       name=f"ps_{d}{b}_{t}")
                        ro0 = oh[:, 0, 0:A]
                        ro1 = oh[:, 1, 0:A]
                        # chunk order in PSUM banks: g, i, f, o — so tanh(g)
                        # fires right after the first 3 matmuls and the DVE
                        # chain starts as early as possible.
                        if t > 0:
                            sp = blk["steps"][t - 1]
                            hoff = sp["off"] + (a4 - sp["a4"])
                            rh = hist[d][:, hoff:hoff + A]
                            for j in range(4):
                                nc.tensor.matmul(ps[:, j, a4:W], WH[d][j], rh,
                                                 start=True, stop=False)
                                nc.tensor.matmul(ps[:, j, a4:W], G0[d][j], ro0,
                                                 start=False, stop=False)
                                nc.tensor.matmul(ps[:, j, a4:W], G1[d][j], ro1,
                                                 start=False, stop=True)
                        else:
                            for j in range(4):
                                nc.tensor.matmul(ps[:, j, a4:W], G0[d][j], ro0,
                                                 start=True, stop=False)
                                nc.tensor.matmul(ps[:, j, a4:W], G1[d][j], ro1,
                                                 start=False, stop=True)
                        nc.scalar.activation(gsb[d][:, 0, a4:W],
                                             ps[:, 0, a4:W], AF.Tanh)
                        nc.scalar.activation(gsb[d][:, 1:3, a4:W],
                                             ps[:, 1:3, a4:W], AF.Sigmoid)
                        nc.scalar.activation(gsb[d][:, 3, a4:W],
                                             ps[:, 3, a4:W], AF.Sigmoid)
                        g_s = gsb[d][:, 0, a4:W]
                        i_s = gsb[d][:, 1, a4:W]
                        f_s = gsb[d][:, 2, a4:W]
                        o_s = gsb[d][:, 3, a4:W]
                        c_s = c_t[d][:, a4:W]
                        if t > 0:
                            nc.vector.tensor_tensor(tig[d][:, a4:W], i_s, g_s,
                                                    op=OP.mult)
                            nc.vector.tensor_tensor(c_s, c_s, f_s, op=OP.mult)
                            nc.vector.tensor_tensor(c_s, c_s, tig[d][:, a4:W],
                                                    op=OP.add)
                        else:
                            nc.vector.tensor_tensor(c_s, i_s, g_s, op=OP.mult)
                        nc.scalar.activation(th[d][:, a4:W], c_s, AF.Tanh)
                        nc.vector.tensor_tensor(hist[d][:, off:off + A], o_s,
                                                th[d][:, a4:W], op=OP.mult)
                for d in "fb":
                    nc.sync.dma_start(
                        hist_d[d][:, blk["base"]:blk["base"] + blk["total"]],
                        hist[d][:, 0:blk["total"]])
    nc.compile()
    return nc


def kernel(char_indices, lengths, emb_table, Wih_f, Whh_f, bih_f, bhh_f,
           Wih_b, Whh_b, bih_b, bhh_b):
    char_indices = np.asarray(char_indices).astype(np.int32)
    lengths = np.asarray(lengths).astype(np.int32)

    # --- stratified length-sorted word assignment ---
    order = np.argsort(lengths, kind="stable")
    perm = [order[k::NCORES] for k in range(NCORES)]      # per-core word ids
    lens = [lengths[p] for p in perm]                     # ascending each
    lens_max = np.max(np.stack(lens), axis=0)
    st = _build_structure(lens_max)
    TOT = st["TOT"]

    # --- per-core char arrays (fwd and per-word-reversed bwd) ---
    cf, cb = [], []
    pos = np.arange(L)[None, :]
    for k in range(NCORES):
        chw = char_indices[perm[k]]                       # [NPC, L]
        lk = lens[k]
        valid = pos < lk[:, None]
        f = np.where(valid, chw, 0)
        rev_idx_c = np.clip(lk[:, None] - 1 - pos, 0, L - 1)
        bwd = np.where(valid, np.take_along_axis(chw, rev_idx_c, axis=1), 0)
        cf.append(f)
        cb.append(bwd)

    # --- one-hot char planes [128, 2, TOT] per (core, dir) ---
    # column gbase+(w-a4) of step (b,t) is word col w's char one-hot:
    # plane c//128, row c%128.
    oh_np = {d: [np.zeros((128, 2, max(TOT, 1)), BF16NP) for _ in range(NCORES)]
             for d in "fb"}
    for b in range(NBLK):
        blk = st["blocks"][b]
        for s in blk["steps"]:
            t, a4, A = s["t"], s["a4"], s["A"]
            gbase = blk["base"] + s["off"]
            cols = np.arange(b * W + a4, (b + 1) * W)     # word columns
            ccol = np.arange(A) + gbase
            for k in range(NCORES):
                for d, arr in (("f", cf[k]), ("b", cb[k])):
                    cvals = arr[cols, t]                  # [A]
                    oh_np[d][k][cvals % 128, cvals // 128, ccol] = 1.0

    # --- weights ---
    emb = np.asarray(emb_table, np.float32)

    def packw(Whh, Wih, bih, bhh):
        # PyTorch gate row order i,f,g,o -> PSUM bank order g,i,f,o
        rows = [slice(256, 384), slice(0, 128), slice(128, 256), slice(384, 512)]
        wh = [np.ascontiguousarray(np.asarray(Whh, np.float32)[r].T) for r in rows]
        G = emb @ np.asarray(Wih, np.float32).T \
            + (np.asarray(bih, np.float32) + np.asarray(bhh, np.float32))[None, :]
        # lhsT[k, m] = G[half*128+k, crow_m]  -> G[half rows, crow cols]
        g0 = [G[0:128, r] for r in rows]
        g1 = [G[128:256, r] for r in rows]
        return wh, g0, g1

    wh_f, g0_f, g1_f = packw(Whh_f, Wih_f, bih_f, bhh_f)
    wh_b, g0_b, g1_b = packw(Whh_b, Wih_b, bih_b, bhh_b)
    w_all = np.concatenate(wh_f + wh_b + g0_f + g1_f + g0_b + g1_b,
                           axis=1).astype(BF16NP)

    nc = _build_program(st)
    in_maps = []
    for k in range(NCORES):
        in_maps.append({
            "wts": w_all,
            "oh_f": oh_np["f"][k], "oh_b": oh_np["b"][k],
        })
    trace = os.environ.get("LSTM_TRACE") == "1"
    res = run_bass_kernel_spmd(nc, in_maps, core_ids=list(range(NCORES)),
                               trace=trace)
    if trace and res.exec_time_ns is not None:
        print(f"HW exec time: {res.exec_time_ns} ns")
        print(f"HW exec time mean: {res.mean_exec_time_ns} ns")
        if res.instructions_and_trace:
            print(f"trace: {res.instructions_and_trace[1]}")

    # --- host-side pick of final states ---
    out = np.zeros((N, 2 * H), np.float32)
    for k in range(NCORES):
        lk = lens[k]
        nz = np.nonzero(lk > 0)[0]
        if len(nz) == 0:
            continue
        r = nz
        bidx = r // W
        tstar = lk[r] - 1
        base = np.array([st["blocks"][b]["base"] for b in bidx])
        off = np.array([st["blocks"][b]["steps"][t]["off"]
                        for b, t in zip(bidx, tstar)])
        a4 = np.array([st["blocks"][b]["steps"][t]["a4"]
                       for b, t in zip(bidx, tstar)])
        posn = base + off + (r % W - a4)
        hf = np.asarray(res.results[k]["hist_f"]).astype(np.float32)
        hb = np.asarray(res.results[k]["hist_b"]).astype(np.float32)
        out[perm[k][r], 0:H] = hf[:, posn].T
        out[perm[k][r], H:2 * H] = hb[:, posn].T
    return out


# revision 7
# speedup vs baseline: 1.3961x; 1.2094x over previous
"""CharBiLSTM embedder on 8 Trainium2 NeuronCores (Bass/Tile).

Strategy
--------
Data-parallel over words (16384 words -> 2048/core, stratified by length so
all cores see the same length profile). Per core:

- Words are sorted ascending by length and split into blocks of W=512
  columns. The LSTM state is kept feature-major: h/c tiles are
  [128 hidden, words] so the recurrent matmul (contraction over H=128) maps
  directly onto the PE without transposes.
- Raggedness: at step t only words with len > t (a suffix of the sorted
  block) are computed; matmul/activation/vector op widths shrink with t.
- Both directions are forward scans: the host reverses each word's chars for
  the backward direction, so both scans process valid chars first; the final
  state of a word is its h at step len-1, which is streamed to HBM as part
  of the h history and picked out host-side.
- The char-embedding lookup + input projection + bias runs on the PE as
  one-hot matmuls: gates += G[char] where G = emb_table @ Wih^T + bias is a
  [256, 512] table held as matmul weights; the host uploads the one-hot
  encoding of the (ragged) char stream as two [128, n] bf16 planes, and two
  K=128 matmuls per gate chunk accumulate the input contribution straight
  into the gate PSUM alongside the Whh recurrence matmul.
- Gates: 4 PSUM chunks (order i,f,o,g) so one Sigmoid activation covers
  i,f,o in a single strided read and one Tanh covers g.

The kernel is JIT-specialized to the *structure* implied by `lengths`
(per-step active widths, block step counts); char values and the selection
of final states are runtime data.
"""

import os
import sys

sys.path.insert(0, "/opt/trn_rl_repo")

import numpy as np
import ml_dtypes

import concourse.bass as bass
import concourse.bacc as bacc
import concourse.tile as tile
import concourse.mybir as mybir
from concourse.bass_utils import run_bass_kernel_spmd

V, E, H = 256, 64, 128
N, L = 16384, 24
NCORES = 8
NPC = N // NCORES          # words per core
W = 512                    # words per block (one PSUM bank per gate chunk)
NBLK = NPC // W
FP32 = mybir.dt.float32
BF16 = mybir.dt.bfloat16
AF = mybir.ActivationFunctionType
OP = mybir.AluOpType
BF16NP = ml_dtypes.bfloat16


def _build_structure(lens_max):
    """Uniform (across cores) control structure from per-rank max lengths."""
    st = {"blocks": [], "TOT": 0}
    for b in range(NBLK):
        bl = lens_max[b * W:(b + 1) * W]
        lmax = int(bl[-1])
        steps = []
        off = 0
        for t in range(lmax):
            a_exact = int(np.searchsorted(bl, t, side="right"))  # count len<=t
            a4 = a_exact & ~3
            steps.append({"t": t, "a4": a4, "A": W - a4, "off": off})
            off += W - a4
        st["blocks"].append({"lmax": lmax, "steps": steps, "total": off,
                             "base": st["TOT"]})
        st["TOT"] += off
    st["MAXTOT"] = max((blk["total"] for blk in st["blocks"]), default=1)
    return st


def _build_program(st):
    nc = bacc.Bacc("TRN2")
    TOT, MAXTOT = st["TOT"], st["MAXTOT"]

    # weights: 24 chunks of [128, 128]: WH f(4) b(4); G f half0(4) half1(4);
    # G b half0(4) half1(4)
    w_d = nc.dram_tensor("wts", [128, 24 * 128], BF16, kind="ExternalInput")
    oh_d = {d: nc.dram_tensor(f"oh_{d}", [128, 2, max(TOT, 1)], BF16,
                              kind="ExternalInput") for d in "fb"}
    hist_d = {d: nc.dram_tensor(f"hist_{d}", [128, max(TOT, 1)], BF16,
                                kind="ExternalOutput") for d in "fb"}

    with tile.TileContext(nc) as tc:
        with (
            tc.tile_pool(name="const", bufs=1) as const_p,
            tc.tile_pool(name="oh", bufs=4) as oh_p,
            tc.tile_pool(name="hist", bufs=2) as hist_p,
            tc.tile_pool(name="state", bufs=1) as state_p,
            tc.tile_pool(name="ps", bufs=1, space="PSUM") as ps_p,
        ):
            w_sb = const_p.tile([128, 24 * 128], BF16)
            nc.sync.dma_start(w_sb[:], w_d[:])
            ch = [w_sb[:, c * 128:(c + 1) * 128] for c in range(24)]
            WH = {"f": ch[0:4], "b": ch[4:8]}
            G0 = {"f": ch[8:12], "b": ch[16:20]}
            G1 = {"f": ch[12:16], "b": ch[20:24]}

            for b in range(NBLK):
                blk = st["blocks"][b]
                if blk["lmax"] == 0:
                    continue
                hist, c_t, th, tig, gsb = {}, {}, {}, {}, {}
                for d in "fb":
                    hist[d] = hist_p.tile([128, MAXTOT], BF16, tag=f"hist_{d}",
                                          name=f"hist_{d}{b}")
                    c_t[d] = state_p.tile([128, W], BF16, tag=f"c_{d}",
                                          name=f"c_{d}{b}")
                    th[d] = state_p.tile([128, W], BF16, tag=f"th_{d}",
                                         name=f"th_{d}{b}")
                    tig[d] = state_p.tile([128, W], BF16, tag=f"tig_{d}",
                                          name=f"tig_{d}{b}")
                    gsb[d] = state_p.tile([128, 4, W], BF16, tag=f"g_{d}",
                                          name=f"g_{d}{b}")
                for t in range(blk["lmax"]):
                    s = blk["steps"][t]
                    a4, A, off = s["a4"], s["A"], s["off"]
                    gbase = blk["base"] + off
                    for d in "fb":
                        oh = oh_p.tile([128, 2, W], BF16, tag=f"oh_{d}",
                                       name=f"oh_{d}{b}_{t}")
                        nc.sync.dma_start(oh[:, :, 0:A],
                                          oh_d[d][:, :, gbase:gbase + A])
                        ps = ps_p.tile([128, 4, W], FP32, tag=f"ps_{d}",
                                       name=f"ps_{d}{b}_{t}")
                        ro0 = oh[:, 0, 0:A]
                        ro1 = oh[:, 1, 0:A]
                        # chunk order in PSUM banks: g, i, f, o — so tanh(g)
                        # fires right after the first 3 matmuls and the DVE
                        # chain starts as early as possible.
                        if t > 0:
                            sp = blk["steps"][t - 1]
                            hoff = sp["off"] + (a4 - sp["a4"])
                            rh = hist[d][:, hoff:hoff + A]
                            # G matmuls first: they don't depend on h(t-1), so
                            # the PE can run them while the previous step's
                            # ACT/DVE tail is still producing h.
                            for j in range(4):
                                nc.tensor.matmul(ps[:, j, a4:W], G0[d][j], ro0,
                                                 start=True, stop=False)
                                nc.tensor.matmul(ps[:, j, a4:W], G1[d][j], ro1,
                                                 start=False, stop=False)
                            for j in range(4):
                                nc.tensor.matmul(ps[:, j, a4:W], WH[d][j], rh,
                                                 start=False, stop=True)
                        else:
                            for j in range(4):
                                nc.tensor.matmul(ps[:, j, a4:W], G0[d][j], ro0,
                                                 start=True, stop=False)
                                nc.tensor.matmul(ps[:, j, a4:W], G1[d][j], ro1,
                                                 start=False, stop=True)
                        nc.scalar.activation(gsb[d][:, 0, a4:W],
                                             ps[:, 0, a4:W], AF.Tanh)
                        nc.scalar.activation(gsb[d][:, 1:3, a4:W],
                                             ps[:, 1:3, a4:W], AF.Sigmoid)
                        nc.scalar.activation(gsb[d][:, 3, a4:W],
                                             ps[:, 3, a4:W], AF.Sigmoid)
                        g_s = gsb[d][:, 0, a4:W]
                        i_s = gsb[d][:, 1, a4:W]
                        f_s = gsb[d][:, 2, a4:W]
                        o_s = gsb[d][:, 3, a4:W]
                        c_s = c_t[d][:, a4:W]
                        if t > 0:
                            nc.vector.tensor_tensor(tig[d][:, a4:W], i_s, g_s,
                                                    op=OP.mult)
                            nc.vector.tensor_tensor(c_s, c_s, f_s, op=OP.mult)
                            nc.vector.tensor_tensor(c_s, c_s, tig[d][:, a4:W],
                                                    op=OP.add)
                        else:
                            nc.vector.tensor_tensor(c_s, i_s, g_s, op=OP.mult)
                        nc.scalar.activation(th[d][:, a4:W], c_s, AF.Tanh)
                        nc.vector.tensor_tensor(hist[d][:, off:off + A], o_s,
                                                th[d][:, a4:W], op=OP.mult)
                for d in "fb":
                    nc.sync.dma_start(
                        hist_d[d][:, blk["base"]:blk["base"] + blk["total"]],
                        hist[d][:, 0:blk["total"]])
    nc.compile()
    return nc


def kernel(char_indices, lengths, emb_table, Wih_f, Whh_f, bih_f, bhh_f,
           Wih_b, Whh_b, bih_b, bhh_b):
    char_indices = np.asarray(char_indices).astype(np.int32)
    lengths = np.asarray(lengths).astype(np.int32)

    # --- stratified length-sorted word assignment ---
    order = np.argsort(lengths, kind="stable")
    perm = [order[k::NCORES] for k in range(NCORES)]      # per-core word ids
    lens = [lengths[p] for p in perm]                     # ascending each
    lens_max = np.max(np.stack(lens), axis=0)
    st = _build_structure(lens_max)
    TOT = st["TOT"]

    # --- per-core char arrays (fwd and per-word-reversed bwd) ---
    cf, cb = [], []
    pos = np.arange(L)[None, :]
    for k in range(NCORES):
        chw = char_indices[perm[k]]                       # [NPC, L]
        lk = lens[k]
        valid = pos < lk[:, None]
        f = np.where(valid, chw, 0)
        rev_idx_c = np.clip(lk[:, None] - 1 - pos, 0, L - 1)
        bwd = np.where(valid, np.take_along_axis(chw, rev_idx_c, axis=1), 0)
        cf.append(f)
        cb.append(bwd)

    # --- one-hot char planes [128, 2, TOT] per (core, dir) ---
    # column gbase+(w-a4) of step (b,t) is word col w's char one-hot:
    # plane c//128, row c%128.
    oh_np = {d: [np.zeros((128, 2, max(TOT, 1)), BF16NP) for _ in range(NCORES)]
             for d in "fb"}
    for b in range(NBLK):
        blk = st["blocks"][b]
        for s in blk["steps"]:
            t, a4, A = s["t"], s["a4"], s["A"]
            gbase = blk["base"] + s["off"]
            cols = np.arange(b * W + a4, (b + 1) * W)     # word columns
            ccol = np.arange(A) + gbase
            for k in range(NCORES):
                for d, arr in (("f", cf[k]), ("b", cb[k])):
                    cvals = arr[cols, t]                  # [A]
                    oh_np[d][k][cvals % 128, cvals // 128, ccol] = 1.0

    # --- weights ---
    emb = np.asarray(emb_table, np.float32)

    def packw(Whh, Wih, bih, bhh):
        # PyTorch gate row order i,f,g,o -> PSUM bank order g,i,f,o
        rows = [slice(256, 384), slice(0, 128), slice(128, 256), slice(384, 512)]
        wh = [np.ascontiguousarray(np.asarray(Whh, np.float32)[r].T) for r in rows]
        G = emb @ np.asarray(Wih, np.float32).T \
            + (np.asarray(bih, np.float32) + np.asarray(bhh, np.float32))[None, :]
        # lhsT[k, m] = G[half*128+k, crow_m]  -> G[half rows, crow cols]
        g0 = [G[0:128, r] for r in rows]
        g1 = [G[128:256, r] for r in rows]
        return wh, g0, g1

    wh_f, g0_f, g1_f = packw(Whh_f, Wih_f, bih_f, bhh_f)
    wh_b, g0_b, g1_b = packw(Whh_b, Wih_b, bih_b, bhh_b)
    w_all = np.concatenate(wh_f + wh_b + g0_f + g1_f + g0_b + g1_b,
                           axis=1).astype(BF16NP)

    nc = _build_program(st)
    in_maps = []
    for k in range(NCORES):
        in_maps.append({
            "wts": w_all,
            "oh_f": oh_np["f"][k], "oh_b": oh_np["b"][k],
        })
    trace = os.environ.get("LSTM_TRACE") == "1"
    res = run_bass_kernel_spmd(nc, in_maps, core_ids=list(range(NCORES)),
                               trace=trace)
    if trace and res.exec_time_ns is not None:
        print(f"HW exec time: {res.exec_time_ns} ns")
        print(f"HW exec time mean: {res.mean_exec_time_ns} ns")
        if res.instructions_and_trace:
            print(f"trace: {res.instructions_and_trace[1]}")

    # --- host-side pick of final states ---
    out = np.zeros((N, 2 * H), np.float32)
    for k in range(NCORES):
        lk = lens[k]
        nz = np.nonzero(lk > 0)[0]
        if len(nz) == 0:
            continue
        r = nz
        bidx = r // W
        tstar = lk[r] - 1
        base = np.array([st["blocks"][b]["base"] for b in bidx])
        off = np.array([st["blocks"][b]["steps"][t]["off"]
                        for b, t in zip(bidx, tstar)])
        a4 = np.array([st["blocks"][b]["steps"][t]["a4"]
                       for b, t in zip(bidx, tstar)])
        posn = base + off + (r % W - a4)
        hf = np.asarray(res.results[k]["hist_f"]).astype(np.float32)
        hb = np.asarray(res.results[k]["hist_b"]).astype(np.float32)
        out[perm[k][r], 0:H] = hf[:, posn].T
        out[perm[k][r], H:2 * H] = hb[:, posn].T
    return out
